# revision 20
# baseline (speedup 1.0000x reference)
"""Canny-style non-max suppression on 8 Trainium2 NeuronCores.

Reference semantics (f32 reproduction):
    deg = theta * f32(180/pi);  deg' = deg + 180 if deg < 0
    k = round_half_even(deg'/45); class: k in {0,4} -> 0deg, 1 -> 45deg,
    2 -> 90deg, else 135deg. mask = img >= both neighbors along class
    direction; out = img*mask on the interior, 0 on the 1-px border.

Device algorithm. With w = |C*theta|, u = |w-90|, z = |u-90|,
b = |C*theta + 45|, u45 = |b-90| (all comparisons against
exactly-representable f32 thresholds):
    is90 <=> u <= 22.5
    is0  <=> z <= 22.5
    is45 <=> u45 < 22.5      (phase-shift identity: |C*theta+45| in
                              (67.5,112.5) <=> deg in (22.5,67.5) or
                              deg in (-157.5,-112.5), incl. the tail
                              handling that matches the reference's
                              else-branch for |deg| > 202.5)
    else 135.
msel = class-selected neighbor-pair max; out = img * (img >= msel).

Engine split per chunk (the DVE is the bottleneck engine):
    ScalarE (8 passes): the 5 angle folds + 3 Relu masks, emitted as
        u32 with a 2^20 pre-scale (Relu(K*(thr - x)) -> u32) so the
        f32->u32 cast can never truncate a selected pixel to 0 and
        negative-zero can never leak a truthy bit pattern.
    VectorE (8 passes): 4 neighbor-pair maxes (tensor_tensor max on
        shifted views), 3 copy_predicated class-selects, 1 fused
        gate (img >= msel ? img : 0).

Sharding: rows split 8 ways; halo handled by passing each core a
1-row/1-col edge-replicated padded img shard (border outputs are zeroed
on the host afterwards, so replicated-edge values never matter).

On-chip layout: partition p holds R0 consecutive image rows (plus a
1-row halo on each side in the img tile), so every one of the 8
neighbor shifts is a pure free-dim AP offset.
"""

import sys

if "/opt/trn_rl_repo" not in sys.path:
    sys.path.insert(0, "/opt/trn_rl_repo")

import numpy as np

import concourse.bass as bass
import concourse.bacc as bacc
import concourse.tile as tile
from concourse import mybir
from concourse.bass_utils import run_bass_kernel_spmd

F32 = mybir.dt.float32
U32 = mybir.dt.uint32
U16 = mybir.dt.uint16
ALU = mybir.AluOpType
ACTF = mybir.ActivationFunctionType


# ---- custom fused DVE op ----------------------------------------------------
# NMS_GATE_ANT: out = (in0 >= in1) ? in0 : 0        (final suppress gate)
from concourse import dve_ops as _dvo
from concourse.dve_spec import (
    Spec as _Spec, Src0 as _S0, Src1 as _S1, Zero as _Z,
    select as _sel, lower as _lower,
)
from concourse.dve_ops import DveOpSpec as _DveOpSpec, has_src1 as _has_src1


def _register(name, spec):
    if name in _dvo._SUB_OPCODE_FOR_NAME:
        return next(o for o in _dvo.OPS if o.name == name)
    row = max(_dvo._SUB_OPCODE_FOR_NAME.values()) + 1
    shas = {
        ver: _DveOpSpec(
            name=name, opcode=row, uops=_lower(spec, ver=ver),
            rd1_en=_has_src1(spec),
        ).sha(ver)
        for ver in ("v3", "v4")
    }
    op = _dvo.DveOp(name, spec, subdim=False, uops_sha=shas)
    _dvo._SUB_OPCODE_FOR_NAME[name] = row
    _dvo.OPS.append(op)
    _dvo.CUSTOM_DVE_SPECS[name] = spec
    return op


def _flat2(a):
    return a.reshape(a.shape[0], -1)


NMS_GATE_ANT = _register(
    "NMS_GATE_ANT",
    _Spec(
        body=_sel(_S0 >= _S1, _S0, _Z),
        reference=lambda in0, in1, s0, s1, imm2: np.where(
            _flat2(in0) >= _flat2(in1), _flat2(in0), 0.0
        ).astype(np.float32),
    ),
)

H = W = 4096
NCORES = 8
SH = H // NCORES  # rows per core (512)

C = float(np.float32(180.0 / np.pi))
EPS225 = float(np.nextafter(np.float32(22.5), np.float32(np.inf)))
MK = float(2.0 ** 11)  # mask pre-scale before the u16 cast (trunc width 2^-11 deg)


def build_nc(
    sh=SH, w=W, wc=512, n_cores=NCORES, reps=1, timing_mode=False, hw_loop=0
):
    """Build the SPMD single-core program (same for all cores).

    reps > 1 repeats the whole (idempotent) computation for differential
    wall-clock timing; the output is identical.
    timing_mode uses internal (untransferred, garbage-data) DRAM tensors so
    wall-clock measures device execution, not host<->device transfer. The
    computation is data-independent, so timing is representative.
    hw_loop > 0 wraps the computation in a device-side For_i loop with that
    trip count (for timing: device time scales with hw_loop, code size not).
    """
    r0 = sh // 128  # rows per partition
    assert sh % 128 == 0 and w % wc == 0
    nchunk = w // wc
    wp = w + 2  # padded img width

    nc = bacc.Bacc(
        "TRN2", target_bir_lowering=False, debug=False, num_devices=n_cores
    )
    if timing_mode:
        img_d = nc.dram_tensor("img", [sh + 2, wp], F32)
        th_d = nc.dram_tensor("theta", [sh, w], F32)
        out_d = nc.dram_tensor("out", [sh, w], F32)
        dummy_d = nc.declare_dram_parameter("tout", [128, 4], F32, isOutput=True)
    else:
        img_d = nc.declare_dram_parameter("img", [sh + 2, wp], F32, isOutput=False)
        th_d = nc.declare_dram_parameter("theta", [sh, w], F32, isOutput=False)
        out_d = nc.declare_dram_parameter("out", [sh, w], F32, isOutput=True)
    img_ap = img_d.ap()
    th_ap = th_d.ap()
    out_ap = out_d.ap()

    v = nc.vector
    s = nc.scalar

    with tile.TileContext(nc) as tc:
        with (
            tc.tile_pool(name="const", bufs=1) as cst,
            tc.tile_pool(name="io", bufs=2) as io,
            tc.tile_pool(name="ang", bufs=2) as ang,
            tc.tile_pool(name="msk", bufs=2) as msk,
            tc.tile_pool(name="dve", bufs=2) as dve,
        ):
            def const_col(val):
                t = cst.tile([128, 1], F32, tag=f"c{val}")
                v.memset(t, val)
                return t

            b45 = const_col(45.0)
            bm90 = const_col(-90.0)
            bm45k = const_col(MK * 22.5)
            bmek = const_col(MK * EPS225)

            import contextlib

            loop_cm = tc.For_i(0, hw_loop, 1) if hw_loop else contextlib.nullcontext()
            with loop_cm:
                for j in range(nchunk * reps):
                    c0 = (j % nchunk) * wc

                    # chunk 0's loads issue from the ACT HWDGE queue: the
                    # ACT stream finishes its masks before the last gate, so
                    # across the For_i back-edge these loads prefetch while
                    # the SP queue is still draining the last stores
                    load_eng = s if (j % nchunk) == 0 else nc.sync
                    img_t = io.tile([128, r0 + 2, wc + 2], F32, tag="img")
                    load_eng.dma_start(
                        out=img_t,
                        in_=bass.AP(
                            tensor=img_ap.tensor,
                            offset=c0,
                            ap=[[r0 * wp, 128], [wp, r0 + 2], [1, wc + 2]],
                        ),
                    )
                    th_t = io.tile([128, r0, wc], F32, tag="theta")
                    load_eng.dma_start(
                        out=th_t,
                        in_=bass.AP(
                            tensor=th_ap.tensor,
                            offset=c0,
                            ap=[[r0 * w, 128], [w, r0], [1, wc]],
                        ),
                    )

                    def ic(dr, dc):  # img neighbor view at (row+dr, col+dc)
                        return img_t[:, 1 + dr : 1 + dr + r0, 1 + dc : 1 + dc + wc]

                    # ---- ScalarE: angle folds + masks ----
                    b_t = ang.tile([128, r0, wc], F32, tag="s1")
                    s.activation(b_t, th_t, ACTF.Abs, scale=C, bias=b45)
                    u45_t = ang.tile([128, r0, wc], F32, tag="s2")
                    s.activation(u45_t, b_t, ACTF.Abs, bias=bm90)
                    m45 = msk.tile([128, r0, wc], U16, tag="m45")
                    s.activation(m45, u45_t, ACTF.Relu, scale=-MK, bias=bm45k)
                    w_t = ang.tile([128, r0, wc], F32, tag="s1")
                    s.activation(w_t, th_t, ACTF.Abs, scale=C)
                    u_t = ang.tile([128, r0, wc], F32, tag="s2")
                    s.activation(u_t, w_t, ACTF.Abs, bias=bm90)
                    m90 = msk.tile([128, r0, wc], U16, tag="m90")
                    s.activation(m90, u_t, ACTF.Relu, scale=-MK, bias=bmek)
                    z_t = ang.tile([128, r0, wc], F32, tag="s1")
                    s.activation(z_t, u_t, ACTF.Abs, bias=bm90)
                    m0 = msk.tile([128, r0, wc], U16, tag="m0")
                    s.activation(m0, z_t, ACTF.Relu, scale=-MK, bias=bmek)

                    # ---- VectorE: neighbor pair maxes as two 2-page TT
                    # ops into one 4-page tile (pages t135, t45, t90, t0) ----
                    def pages(view, stride, n):
                        lst = [list(x) for x in view.ap]
                        return bass.AP(
                            tensor=view.tensor,
                            offset=view.offset,
                            ap=[lst[0], [stride, n]] + lst[1:],
                        )

                    big = dve.tile([128, 4, r0, wc], F32, tag="big")
                    v.tensor_tensor(
                        big[:, 0:2], pages(ic(1, -1), 2, 2),
                        pages(ic(-1, 1), -2, 2), ALU.max,
                    )
                    v.tensor_tensor(
                        big[:, 2:4], pages(ic(-1, 0), wc + 1, 2),
                        pages(ic(1, 0), -(wc + 1), 2), ALU.max,
                    )
                    msel = big[:, 0]

                    # ---- class-select the neighbor max ----
                    v.copy_predicated(msel, m45, big[:, 1])
                    v.copy_predicated(msel, m90, big[:, 2])
                    v.copy_predicated(msel, m0, big[:, 3])

                    # ---- out = (img >= msel) ? img : 0 ----
                    out_t = io.tile([128, r0, wc], F32, tag="out")
                    v._custom_dve(NMS_GATE_ANT, out=out_t, in0=ic(0, 0), in1=msel)

                    nc.sync.dma_start(
                        out=bass.AP(
                            tensor=out_ap.tensor,
                            offset=c0,
                            ap=[[r0 * w, 128], [w, r0], [1, wc]],
                        ),
                        in_=out_t,
                    )
            if timing_mode:
                nc.sync.dma_start(out=dummy_d.ap(), in_=out_t[:, 0, 0:4])
    nc.compile()
    return nc


def run(img2d, theta2d, sh=SH, wc=512, trace=False):
    """img2d/theta2d: full (H', W) f32 arrays with H' = 8*sh."""
    h, w = img2d.shape
    n_cores = NCORES
    assert h == n_cores * sh and theta2d.shape == (h, w)
    imgp = np.pad(img2d, 1, mode="edge")
    in_maps = [
        {
            "img": np.ascontiguousarray(imgp[k * sh : k * sh + sh + 2, :]),
            "theta": np.ascontiguousarray(theta2d[k * sh : (k + 1) * sh, :]),
        }
        for k in range(n_cores)
    ]
    nc = build_nc(sh=sh, w=w, wc=wc, n_cores=n_cores)
    res = run_bass_kernel_spmd(nc, in_maps, list(range(n_cores)), trace=trace)
    out = np.concatenate([res.results[k]["out"] for k in range(n_cores)], axis=0)
    out[0, :] = 0
    out[-1, :] = 0
    out[:, 0] = 0
    out[:, -1] = 0
    return out, res


def kernel(img: np.ndarray, theta: np.ndarray) -> np.ndarray:
    img2d = np.asarray(img, dtype=np.float32).reshape(H, W)
    th2d = np.asarray(theta, dtype=np.float32).reshape(H, W)
    out, _ = run(img2d, th2d)
    return out.reshape(1, 1, H, W)


# revision 21
# speedup vs baseline: 1.0056x; 1.0056x over previous
"""Canny-style non-max suppression on 8 Trainium2 NeuronCores.

Reference semantics (f32 reproduction):
    deg = theta * f32(180/pi);  deg' = deg + 180 if deg < 0
    k = round_half_even(deg'/45); class: k in {0,4} -> 0deg, 1 -> 45deg,
    2 -> 90deg, else 135deg. mask = img >= both neighbors along class
    direction; out = img*mask on the interior, 0 on the 1-px border.

Device algorithm. With w = |C*theta|, u = |w-90|, z = |u-90|,
b = |C*theta + 45|, u45 = |b-90| (all comparisons against
exactly-representable f32 thresholds):
    is90 <=> u <= 22.5
    is0  <=> z <= 22.5
    is45 <=> u45 < 22.5      (phase-shift identity: |C*theta+45| in
                              (67.5,112.5) <=> deg in (22.5,67.5) or
                              deg in (-157.5,-112.5), incl. the tail
                              handling that matches the reference's
                              else-branch for |deg| > 202.5)
    else 135.
msel = class-selected neighbor-pair max; out = img * (img >= msel).

Engine split per chunk (the DVE is the bottleneck engine):
    ScalarE (8 passes): the 5 angle folds + 3 Relu masks, emitted as
        u16 with a 2^11 pre-scale (Relu(K*(thr - x)) -> u16) so the
        float->uint cast keeps band-interior pixels nonzero (the cast
        truncation only loses a ~2^-11 deg sliver at each band edge,
        ~24 px on the 4k x 4k image) and negative-zero can never leak
        a truthy bit pattern.
    VectorE (6 instructions, 8 element-passes): the 4 neighbor-pair
        maxes run as two 2-page tensor_tensor max instructions into a
        4-page tile (page strides reach the paired shifted views), then
        3 copy_predicated class-selects, then the fused gate
        (img >= msel ? img : 0).
    Found experimentally: per-op costs are dominated by scheduling
    effects, not the cost model. Stock-op streams run at ~(N+100)/0.96
    ns; a custom DVE op mixed into a stock stream costs ~3.7us extra
    per transition in isolation but overlaps with cross-engine slack
    here; manual software pipelining / load-ahead emission fights the
    Tile list-scheduler and regresses badly (255us vs 170us).

Sharding: rows split 8 ways; halo handled by passing each core a
1-row/1-col edge-replicated padded img shard (border outputs are zeroed
on the host afterwards, so replicated-edge values never matter).

On-chip layout: partition p holds R0 consecutive image rows (plus a
1-row halo on each side in the img tile), so every one of the 8
neighbor shifts is a pure free-dim AP offset.
"""

import sys

if "/opt/trn_rl_repo" not in sys.path:
    sys.path.insert(0, "/opt/trn_rl_repo")

import numpy as np

import concourse.bass as bass
import concourse.bacc as bacc
import concourse.tile as tile
from concourse import mybir
from concourse.bass_utils import run_bass_kernel_spmd

F32 = mybir.dt.float32
U32 = mybir.dt.uint32
U16 = mybir.dt.uint16
ALU = mybir.AluOpType
ACTF = mybir.ActivationFunctionType


# ---- custom fused DVE op ----------------------------------------------------
# NMS_GATE_ANT: out = (in0 >= in1) ? in0 : 0        (final suppress gate)
from concourse import dve_ops as _dvo
from concourse.dve_spec import (
    Spec as _Spec, Src0 as _S0, Src1 as _S1, Zero as _Z,
    select as _sel, lower as _lower,
)
from concourse.dve_ops import DveOpSpec as _DveOpSpec, has_src1 as _has_src1


def _register(name, spec):
    if name in _dvo._SUB_OPCODE_FOR_NAME:
        return next(o for o in _dvo.OPS if o.name == name)
    row = max(_dvo._SUB_OPCODE_FOR_NAME.values()) + 1
    shas = {
        ver: _DveOpSpec(
            name=name, opcode=row, uops=_lower(spec, ver=ver),
            rd1_en=_has_src1(spec),
        ).sha(ver)
        for ver in ("v3", "v4")
    }
    op = _dvo.DveOp(name, spec, subdim=False, uops_sha=shas)
    _dvo._SUB_OPCODE_FOR_NAME[name] = row
    _dvo.OPS.append(op)
    _dvo.CUSTOM_DVE_SPECS[name] = spec
    return op


def _flat2(a):
    return a.reshape(a.shape[0], -1)


NMS_GATE_ANT = _register(
    "NMS_GATE_ANT",
    _Spec(
        body=_sel(_S0 >= _S1, _S0, _Z),
        reference=lambda in0, in1, s0, s1, imm2: np.where(
            _flat2(in0) >= _flat2(in1), _flat2(in0), 0.0
        ).astype(np.float32),
    ),
)

H = W = 4096
NCORES = 8
SH = H // NCORES  # rows per core (512)

C = float(np.float32(180.0 / np.pi))
EPS225 = float(np.nextafter(np.float32(22.5), np.float32(np.inf)))
MK = float(2.0 ** 11)  # mask pre-scale before the u16 cast (trunc width 2^-11 deg)


def build_nc(
    sh=SH, w=W, wc=512, n_cores=NCORES, reps=1, timing_mode=False, hw_loop=0
):
    """Build the SPMD single-core program (same for all cores).

    reps > 1 repeats the whole (idempotent) computation for differential
    wall-clock timing; the output is identical.
    timing_mode uses internal (untransferred, garbage-data) DRAM tensors so
    wall-clock measures device execution, not host<->device transfer. The
    computation is data-independent, so timing is representative.
    hw_loop > 0 wraps the computation in a device-side For_i loop with that
    trip count (for timing: device time scales with hw_loop, code size not).
    """
    r0 = sh // 128  # rows per partition
    assert sh % 128 == 0 and w % wc == 0
    nchunk = w // wc
    wp = w + 2  # padded img width

    nc = bacc.Bacc(
        "TRN2", target_bir_lowering=False, debug=False, num_devices=n_cores
    )
    if timing_mode:
        img_d = nc.dram_tensor("img", [sh + 2, wp], F32)
        th_d = nc.dram_tensor("theta", [sh, w], F32)
        out_d = nc.dram_tensor("out", [sh, w], F32)
        dummy_d = nc.declare_dram_parameter("tout", [128, 4], F32, isOutput=True)
    else:
        img_d = nc.declare_dram_parameter("img", [sh + 2, wp], F32, isOutput=False)
        th_d = nc.declare_dram_parameter("theta", [sh, w], F32, isOutput=False)
        out_d = nc.declare_dram_parameter("out", [sh, w], F32, isOutput=True)
    img_ap = img_d.ap()
    th_ap = th_d.ap()
    out_ap = out_d.ap()

    v = nc.vector
    s = nc.scalar

    with tile.TileContext(nc) as tc:
        with (
            tc.tile_pool(name="const", bufs=1) as cst,
            tc.tile_pool(name="io", bufs=2) as io,
            tc.tile_pool(name="ang", bufs=2) as ang,
            tc.tile_pool(name="msk", bufs=2) as msk,
            tc.tile_pool(name="dve", bufs=2) as dve,
        ):
            def const_col(val):
                t = cst.tile([128, 1], F32, tag=f"c{val}")
                v.memset(t, val)
                return t

            b45 = const_col(45.0)
            bm90 = const_col(-90.0)
            bm45k = const_col(MK * 22.5)
            bmek = const_col(MK * EPS225)

            import contextlib

            loop_cm = tc.For_i(0, hw_loop, 1) if hw_loop else contextlib.nullcontext()
            with loop_cm:
                for j in range(nchunk * reps):
                    c0 = (j % nchunk) * wc

                    img_t = io.tile([128, r0 + 2, wc + 2], F32, tag="img")
                    nc.sync.dma_start(
                        out=img_t,
                        in_=bass.AP(
                            tensor=img_ap.tensor,
                            offset=c0,
                            ap=[[r0 * wp, 128], [wp, r0 + 2], [1, wc + 2]],
                        ),
                    )
                    th_t = io.tile([128, r0, wc], F32, tag="theta")
                    nc.sync.dma_start(
                        out=th_t,
                        in_=bass.AP(
                            tensor=th_ap.tensor,
                            offset=c0,
                            ap=[[r0 * w, 128], [w, r0], [1, wc]],
                        ),
                    )

                    def ic(dr, dc):  # img neighbor view at (row+dr, col+dc)
                        return img_t[:, 1 + dr : 1 + dr + r0, 1 + dc : 1 + dc + wc]

                    # ---- ScalarE: angle folds + masks ----
                    b_t = ang.tile([128, r0, wc], F32, tag="s1")
                    s.activation(b_t, th_t, ACTF.Abs, scale=C, bias=b45)
                    u45_t = ang.tile([128, r0, wc], F32, tag="s2")
                    s.activation(u45_t, b_t, ACTF.Abs, bias=bm90)
                    m45 = msk.tile([128, r0, wc], U16, tag="m45")
                    s.activation(m45, u45_t, ACTF.Relu, scale=-MK, bias=bm45k)
                    w_t = ang.tile([128, r0, wc], F32, tag="s1")
                    s.activation(w_t, th_t, ACTF.Abs, scale=C)
                    u_t = ang.tile([128, r0, wc], F32, tag="s2")
                    s.activation(u_t, w_t, ACTF.Abs, bias=bm90)
                    m90 = msk.tile([128, r0, wc], U16, tag="m90")
                    s.activation(m90, u_t, ACTF.Relu, scale=-MK, bias=bmek)
                    z_t = ang.tile([128, r0, wc], F32, tag="s1")
                    s.activation(z_t, u_t, ACTF.Abs, bias=bm90)
                    m0 = msk.tile([128, r0, wc], U16, tag="m0")
                    s.activation(m0, z_t, ACTF.Relu, scale=-MK, bias=bmek)

                    # ---- VectorE: neighbor pair maxes as two 2-page TT
                    # ops into one 4-page tile (pages t135, t45, t90, t0) ----
                    def pages(view, stride, n):
                        lst = [list(x) for x in view.ap]
                        return bass.AP(
                            tensor=view.tensor,
                            offset=view.offset,
                            ap=[lst[0], [stride, n]] + lst[1:],
                        )

                    big = dve.tile([128, 4, r0, wc], F32, tag="big")
                    v.tensor_tensor(
                        big[:, 0:2], pages(ic(1, -1), 2, 2),
                        pages(ic(-1, 1), -2, 2), ALU.max,
                    )
                    v.tensor_tensor(
                        big[:, 2:4], pages(ic(-1, 0), wc + 1, 2),
                        pages(ic(1, 0), -(wc + 1), 2), ALU.max,
                    )
                    msel = big[:, 0]

                    # ---- class-select the neighbor max ----
                    v.copy_predicated(msel, m45, big[:, 1])
                    v.copy_predicated(msel, m90, big[:, 2])
                    v.copy_predicated(msel, m0, big[:, 3])

                    # ---- out = (img >= msel) ? img : 0 ----
                    out_t = io.tile([128, r0, wc], F32, tag="out")
                    v._custom_dve(NMS_GATE_ANT, out=out_t, in0=ic(0, 0), in1=msel)

                    nc.sync.dma_start(
                        out=bass.AP(
                            tensor=out_ap.tensor,
                            offset=c0,
                            ap=[[r0 * w, 128], [w, r0], [1, wc]],
                        ),
                        in_=out_t,
                    )
            if timing_mode:
                nc.sync.dma_start(out=dummy_d.ap(), in_=out_t[:, 0, 0:4])
    nc.compile()
    return nc


def run(img2d, theta2d, sh=SH, wc=512, trace=False):
    """img2d/theta2d: full (H', W) f32 arrays with H' = 8*sh."""
    h, w = img2d.shape
    n_cores = NCORES
    assert h == n_cores * sh and theta2d.shape == (h, w)
    imgp = np.pad(img2d, 1, mode="edge")
    in_maps = [
        {
            "img": np.ascontiguousarray(imgp[k * sh : k * sh + sh + 2, :]),
            "theta": np.ascontiguousarray(theta2d[k * sh : (k + 1) * sh, :]),
        }
        for k in range(n_cores)
    ]
    nc = build_nc(sh=sh, w=w, wc=wc, n_cores=n_cores)
    res = run_bass_kernel_spmd(nc, in_maps, list(range(n_cores)), trace=trace)
    out = np.concatenate([res.results[k]["out"] for k in range(n_cores)], axis=0)
    out[0, :] = 0
    out[-1, :] = 0
    out[:, 0] = 0
    out[:, -1] = 0
    return out, res


def kernel(img: np.ndarray, theta: np.ndarray) -> np.ndarray:
    img2d = np.asarray(img, dtype=np.float32).reshape(H, W)
    th2d = np.asarray(theta, dtype=np.float32).reshape(H, W)
    out, _ = run(img2d, th2d)
    return out.reshape(1, 1, H, W)
